# revision 28
# baseline (speedup 1.0000x reference)
"""AttnBlock (GroupNorm + single-head self-attention + residual) on 8 NeuronCores.

Sharding: data-parallel over B (4 batches) x sequence-parallel over query
rows (2 halves of H*W=4096) = 8 shards, one per core.  Each core loads its
batch's full x[b] as [C=128, HW=4096] fp16 (channels on partitions), with
the spatial columns rotated so the core's query half is cols [0:2048)
(attention is permutation-invariant over keys so K/V order is irrelevant).

GroupNorm is folded into the projections: h = A*x + B (per-channel affine
from the group statistics), so only the [128,128] weight scalings depend on
the statistics.  The k-bias is dropped entirely: it shifts every key score
of a given query by the same amount, which softmax cancels.  The V-bias is
folded into an output bias (softmax rows sum to 1).

Main loop: the core's 2048 queries run as two sequential 1024-query passes,
which shrinks the PV accumulator to 2 PSUM banks and leaves three
independent [128,1024] score slots (2 banks each) to triple-buffer the
PE->exp pipeline.  exp work is split three ways: ScalarE exponentiates 3 of
every 4 key blocks ((1024+352)/1.2 ~ 1.1us each); VectorE handles the
fourth with a clamped Schraudolph bit-trick exp (t = A*s + B computed in
fp16, clamped at 0, truncated to int16, bitcast to fp16 ~ exp(s-8) within
+-3.6%); the softmax denominator accumulates in fp16 on two independent
engine-local chains (VectorE for 2 of 3 blocks, GpSimd for the third) so no
cross-engine semaphore hop sits on an accumulation chain.  Each half's
softmax epilogue (column-reduce via PE, reciprocal, selector-matmul
broadcast, normalize straight out of PSUM, Wo projection, residual)
overlaps the other half's loop.
"""

import numpy as np

C = 128
HW = 4096
NQ = 2048  # queries per core
HALF = 1024  # queries per sequential pass
JB = 32  # key blocks of 128
EXP_BIAS = -8.0
EPS = 1e-5
N_CORES = 8
N_WARM = 10  # dummy matmuls to lift the PE HAM clock-gate before real work

# Schraudolph fp16 exp: bitcast(int16(max(SCH_A*s + SCH_B, 0))) ~ exp(s-8)
SCH_A = 1024.0 / float(np.log(2.0))
SCH_B = 15360.0 - 46.0 + SCH_A * EXP_BIAS
SCH_AINV = 1.0 / SCH_A

# wpack (f16) column offsets
_WQ, _WK, _WV, _WO = 0, 128, 256, 384
_GMAP, _ONESC = 512, 544
_GMAPT, _SEL8 = 546, 674
_WPACK_W = 674 + 8 * 128
# fpack (f32) column offsets
_NW, _NWN, _NB, _IDENT = 0, 1, 2, 3
_FPACK_W = 131

_NC = None


def _pin_activation_tables():
    """Restrict the table-load chooser to natural_log_exp_and_others so the
    kernel's ACT stream (copy/identity/ln/exp) needs a single table load."""
    from concourse.hw_specs import get_activation_tables
    tabs = get_activation_tables("gen3")
    for name in list(tabs.keys()):
        if name != "natural_log_exp_and_others":
            tabs[name] = set()


def _build_program():
    import concourse.bacc as bacc
    import concourse.tile as tile
    from concourse import mybir

    f32 = mybir.dt.float32
    f16 = mybir.dt.float16
    i16 = mybir.dt.int16
    AF = mybir.ActivationFunctionType
    OP = mybir.AluOpType

    nc = bacc.Bacc("TRN2", target_bir_lowering=False, debug=False,
                   num_devices=N_CORES)
    try:
        _pin_activation_tables()
    except Exception:
        pass

    x_d = nc.declare_dram_parameter("x", [C, HW], f16, isOutput=False)
    wpack_d = nc.declare_dram_parameter("wpack", [C, _WPACK_W], f16,
                                        isOutput=False)
    fpack_d = nc.declare_dram_parameter("fpack", [C, _FPACK_W], f32,
                                        isOutput=False)
    y_d = nc.declare_dram_parameter("y", [C, NQ], f16, isOutput=True)

    with tile.TileContext(nc) as tc:
        consts = tc.alloc_tile_pool(name="consts", bufs=1)
        big = tc.alloc_tile_pool(name="big", bufs=1)
        work = tc.alloc_tile_pool(name="work", bufs=3)
        epool = tc.alloc_tile_pool(name="epool", bufs=4)   # [C,1024] E tiles
        spool = tc.alloc_tile_pool(name="spool", bufs=2)   # Schraudolph tmp
        accs = tc.alloc_tile_pool(name="accs", bufs=1)
        ypool = tc.alloc_tile_pool(name="ypool", bufs=4)
        # PSUM: 8 banks = 3 score/scratch slots (2 each) + oT (2)
        ps = tc.alloc_tile_pool(name="ps", bufs=3, space="PSUM")
        pot = tc.alloc_tile_pool(name="pot", bufs=1, space="PSUM")

        # ---- input DMAs first: x in 4 chunks of 1024 cols (2KB/partition
        # lines), one per engine ring for queue-level parallelism.
        x16 = big.tile([C, HW], f16)
        for ch, eng in enumerate((nc.sync, nc.scalar, nc.gpsimd, nc.sync)):
            eng.dma_start(out=x16[:, ch * 1024:(ch + 1) * 1024],
                          in_=x_d.ap()[:, ch * 1024:(ch + 1) * 1024])
        wpack_sb = consts.tile([C, _WPACK_W], f16)
        nc.gpsimd.dma_start(out=wpack_sb, in_=wpack_d.ap())
        fpack_sb = consts.tile([C, _FPACK_W], f32)
        nc.gpsimd.dma_start(out=fpack_sb, in_=fpack_d.ap())
        wq_sb = wpack_sb[:, _WQ:_WQ + C]
        wk_sb = wpack_sb[:, _WK:_WK + C]
        wv_sb = wpack_sb[:, _WV:_WV + C]
        wo_sb = wpack_sb[:, _WO:_WO + C]
        gmap_sb = wpack_sb[:, _GMAP:_GMAP + 32]
        onesc_sb = wpack_sb[:, _ONESC:_ONESC + 1]
        gmapt_sb = wpack_sb[0:32, _GMAPT:_GMAPT + C]
        sel8_sb = wpack_sb[0:8, _SEL8:_SEL8 + 8 * C]
        nw_sb = fpack_sb[:, _NW:_NW + 1]
        nwn_sb = fpack_sb[:, _NWN:_NWN + 1]
        nb_sb = fpack_sb[:, _NB:_NB + 1]
        ident_sb = fpack_sb[:, _IDENT:_IDENT + C]
        eps_sb = consts.tile([32, 1], f32)
        nc.vector.memset(eps_sb, EPS)
        ebias_sb = consts.tile([C, 1], f32)
        nc.vector.memset(ebias_sb, EXP_BIAS)

        # ---- PE warmup: back-to-back dummy matmuls so the HAM clock-gate
        # reaches K=8/8 (2.4 GHz) before the real matmul stream starts.
        wz = consts.tile([C, 512], f16)
        nc.vector.memset(wz, 0.0)
        warm_ps = ps.tile([C, 1024], f32, tag="ps", name="warm0")
        for _ in range(N_WARM):
            nc.tensor.matmul(out=warm_ps[:, 0:512], lhsT=wz[:, 0:C],
                             rhs=wz)

        # ---- GroupNorm stats.  Chunk-gated dummy matmul bursts keep the
        # HAM activity window busy through the DMA/stats phase.
        stats = work.tile([C, 8, 6], f32)
        for ch in range(4):
            for h in range(2):
                nc.vector.bn_stats(
                    out=stats[:, 2 * ch + h, :],
                    in_=x16[:, ch * 1024 + h * 512:ch * 1024 + (h + 1) * 512])
            wp = ps.tile([C, 1024], f32, tag="ps", name=f"warmx{ch}")
            for k in range(4):
                nc.tensor.matmul(out=wp[:, 0:512],
                                 lhsT=x16[:, ch * 1024:ch * 1024 + C],
                                 rhs=x16[:, ch * 1024:ch * 1024 + 512])
        mv = work.tile([C, 2], f32)
        nc.vector.bn_aggr(out=mv, in_=stats)
        # spack = (mean, E[x^2]) packed f16; E[x^2] = mean^2 + var in one op
        spack = work.tile([C, 2], f16)
        nc.vector.tensor_scalar(out=spack[:, 1:2], in0=mv[:, 0:1],
                                scalar1=mv[:, 0:1], scalar2=mv[:, 1:2],
                                op0=OP.mult, op1=OP.add)
        nc.vector.tensor_copy(out=spack[:, 0:1], in_=mv[:, 0:1])
        # gmap carries the 0.25 group-average fold
        gsum = ps.tile([32, 2], f32, tag="ps", name="gsum")
        nc.tensor.matmul(out=gsum, lhsT=gmap_sb, rhs=spack)
        # keep HAM busy across the bn_aggr -> gsum hop (behind gsum in the
        # PE FIFO so the filler never delays the chain)
        wpa = ps.tile([C, 1024], f32, tag="ps", name="warma")
        for _ in range(4):
            nc.tensor.matmul(out=wpa[:, 0:512], lhsT=wz[:, 0:C], rhs=wz)
        # -gvar = gm^2 - gex2 in one op; rstd = exp(-0.5*ln(eps - (-gvar)))
        ngvar = work.tile([32, 1], f32)
        nc.vector.tensor_scalar(out=ngvar, in0=gsum[:, 0:1],
                                scalar1=gsum[:, 0:1], scalar2=gsum[:, 1:2],
                                op0=OP.mult, op1=OP.subtract)
        gln = work.tile([32, 1], f32)
        nc.scalar.activation(out=gln, in_=ngvar, func=AF.Ln, bias=eps_sb,
                             scale=-1.0)
        grs = work.tile([32, 1], f32)
        nc.scalar.activation(out=grs, in_=gln, func=AF.Exp, scale=-0.5)
        gpack = work.tile([32, 2], f16)
        nc.vector.tensor_copy(out=gpack[:, 0:1], in_=gsum[:, 0:1])
        nc.vector.tensor_copy(out=gpack[:, 1:2], in_=grs)
        cstat = ps.tile([C, 2], f32, tag="ps", name="cstat")
        nc.tensor.matmul(out=cstat, lhsT=gmapt_sb, rhs=gpack)
        # filler burst behind cstat keeps HAM at 8/8 through the chain
        wpd = ps.tile([C, 1024], f32, tag="ps", name="warmd")
        for _ in range(4):
            nc.tensor.matmul(out=wpd[:, 0:512], lhsT=wz[:, 0:C], rhs=wz)
        affA = work.tile([C, 1], f32)
        nc.vector.tensor_mul(out=affA, in0=cstat[:, 1:2], in1=nw_sb)
        affAn = work.tile([C, 1], f32)
        nc.vector.tensor_mul(out=affAn, in0=cstat[:, 1:2], in1=nwn_sb)
        # affB = nb + cstat0 * (-affA)
        affB = work.tile([C, 1], f32)
        nc.vector.scalar_tensor_tensor(out=affB, in0=cstat[:, 0:1],
                                       scalar=affAn, in1=nb_sb,
                                       op0=OP.mult, op1=OP.add)
        b16 = work.tile([C, 1], f16)
        nc.vector.tensor_copy(out=b16, in_=affB)

        # ---- fold affine scale into projection weights; biases via tiny MMs
        wqa = consts.tile([C, C], f16)
        nc.vector.tensor_scalar_mul(out=wqa, in0=wq_sb, scalar1=affA)
        wka = consts.tile([C, C], f16)
        nc.vector.tensor_scalar_mul(out=wka, in0=wk_sb, scalar1=affA)
        wva = consts.tile([C, C], f16)
        nc.vector.tensor_scalar_mul(out=wva, in0=wv_sb, scalar1=affA)
        pb = ps.tile([C, 2], f32, tag="ps", name="pb")
        nc.tensor.matmul(out=pb[:, 0:1], lhsT=wq_sb, rhs=b16)
        nc.tensor.matmul(out=pb[:, 1:2], lhsT=wv_sb, rhs=b16)
        qb_sb = work.tile([C, 1], f32)
        nc.vector.tensor_copy(out=qb_sb, in_=pb[:, 0:1])
        vb16 = work.tile([C, 1], f16)
        nc.vector.tensor_copy(out=vb16, in_=pb[:, 1:2])
        pob = ps.tile([C, 1], f32, tag="ps", name="pob")
        nc.tensor.matmul(out=pob, lhsT=wo_sb, rhs=vb16)
        obias_sb = work.tile([C, 1], f32)
        nc.vector.tensor_copy(out=obias_sb, in_=pob)

        # ---- projections in [C,1024] slabs.  Loop-start needs qT half0 +
        # kT slab0 (+ V slab0 shortly after); those go first, the remaining
        # slabs are emitted interleaved with the first loop groups.
        # Evictions: Q on ACT (bias), K on DVE, V on GpSimd.
        qT = big.tile([C, NQ], f16)
        kT = big.tile([C, HW], f16)
        v_sb = big.tile([C, HW], f16)  # col block jb holds V0[j, c] rows

        def proj_q(half):
            qps = ps.tile([C, 1024], f32, tag="ps", name=f"qps{half}")
            for k in range(2):
                nc.tensor.matmul(
                    out=qps[:, k * 512:(k + 1) * 512], lhsT=wqa,
                    rhs=x16[:, half * 1024 + k * 512:half * 1024 + (k + 1) * 512])
            nc.scalar.activation(out=qT[:, half * 1024:(half + 1) * 1024],
                                 in_=qps, func=AF.Identity, bias=qb_sb)

        def proj_k(sl):
            kps = ps.tile([C, 1024], f32, tag="ps", name=f"kps{sl}")
            for k in range(2):
                nc.tensor.matmul(
                    out=kps[:, k * 512:(k + 1) * 512], lhsT=wka,
                    rhs=x16[:, sl * 1024 + k * 512:sl * 1024 + (k + 1) * 512])
            nc.vector.tensor_copy(out=kT[:, sl * 1024:(sl + 1) * 1024],
                                  in_=kps)

        def proj_v(sl):
            vps = ps.tile([C, 1024], f32, tag="ps", name=f"vps{sl}")
            for k in range(8):
                jb = sl * 8 + k
                nc.tensor.matmul(out=vps[:, k * 128:(k + 1) * 128],
                                 lhsT=x16[:, jb * 128:(jb + 1) * 128],
                                 rhs=wva)
            if sl % 2 == 0:
                nc.scalar.copy(out=v_sb[:, sl * 1024:(sl + 1) * 1024],
                               in_=vps)
            else:
                nc.vector.tensor_copy(out=v_sb[:, sl * 1024:(sl + 1) * 1024],
                                      in_=vps)

        proj_q(0)
        proj_k(0)
        proj_v(0)

        # ---- main attention loop: two sequential 1024-query passes, 32 key
        # blocks each, software-pipelined by one block (scores jb+1 issue
        # before PV jb).  exp: jb%4==3 -> VectorE Schraudolph, else ScalarE.
        # denominator chains: jb%3==2 -> GpSimd accumulator, else VectorE
        # (two sub-accumulators to break the latency chain).
        def emit_half(half, interleave):
            q0 = half * HALF
            qs = qT[:, q0:q0 + HALF]
            # acc2 halves hold even-jb / odd-jb partial sums
            acc2 = accs.tile([C, 2 * HALF], f16, name=f"acc2_{half}")
            oT = pot.tile([C, HALF], f32, tag="ot", name=f"oT{half}")
            pend = []
            e2 = None
            for jb in range(JB):
                st = ps.tile([C, HALF], f32, tag="ps", name=f"st{half}_{jb}")
                for k in range(2):
                    nc.tensor.matmul(out=st[:, k * 512:(k + 1) * 512],
                                     lhsT=kT[:, jb * 128:(jb + 1) * 128],
                                     rhs=qs[:, k * 512:(k + 1) * 512])
                if jb % 2 == 0:
                    e2 = epool.tile([C, 2 * HALF], f16, tag="e",
                                    name=f"e{half}_{jb}")
                e_t = e2[:, (jb % 2) * HALF:(jb % 2 + 1) * HALF]
                if jb % 6 == 3:
                    tt = spool.tile([C, HALF], f16, tag="s",
                                    name=f"sch{half}_{jb}")
                    nc.vector.tensor_scalar(out=tt, in0=st,
                                            scalar1=SCH_A, scalar2=SCH_B,
                                            op0=OP.mult, op1=OP.add)
                    nc.vector.tensor_scalar_max(out=e_t.bitcast(i16),
                                                in0=tt, scalar1=0.0)
                else:
                    nc.scalar.activation(out=e_t, in_=st, func=AF.Exp,
                                         bias=ebias_sb)
                # one paired denominator accumulation per two blocks
                if jb % 2 == 1:
                    if jb == 1:
                        nc.vector.tensor_copy(out=acc2, in_=e2)
                    else:
                        nc.vector.tensor_add(out=acc2, in0=acc2, in1=e2)
                # PV runs two blocks behind exp so the PE never waits on it
                if len(pend) == 2:
                    pj, pe = pend.pop(0)
                    for k in range(2):
                        nc.tensor.matmul(
                            out=oT[:, k * 512:(k + 1) * 512],
                            lhsT=v_sb[:, pj * 128:(pj + 1) * 128],
                            rhs=pe[:, k * 512:(k + 1) * 512],
                            start=(pj == 0), stop=(pj == JB - 1))
                pend.append((jb, e_t))
                # previous half's epilogue / projection slabs trickle in
                if interleave and jb % 2 == 1:
                    interleave.pop(0)()
            for pj, pe in pend:
                for k in range(2):
                    nc.tensor.matmul(out=oT[:, k * 512:(k + 1) * 512],
                                     lhsT=v_sb[:, pj * 128:(pj + 1) * 128],
                                     rhs=pe[:, k * 512:(k + 1) * 512],
                                     start=(pj == 0), stop=(pj == JB - 1))
            return acc2, oT

        def epilogue_steps(half, acc2, oT):
            """Return the softmax epilogue as closures; step 0 frees oT."""
            q0 = half * HALF
            esS = work.tile([C, HALF], f16, name=f"esS{half}")
            r_col = work.tile([C, 8], f32, name=f"rcol{half}")
            r8_sb = work.tile([8, C], f16, name=f"r8sb{half}")
            oc16 = work.tile([C, HALF], f16, name=f"oc16_{half}")
            onrm = work.tile([C, HALF], f16, name=f"onrm{half}")
            box = {}

            def s0():
                nc.scalar.copy(out=oc16, in_=oT)  # unnormalized; frees oT
                nc.vector.tensor_add(out=esS, in0=acc2[:, 0:HALF],
                                     in1=acc2[:, HALF:2 * HALF])

            def s1():
                scol = ps.tile([C, 8], f32, tag="ps", name=f"scol{half}")
                for ib in range(8):
                    nc.tensor.matmul(out=scol[:, ib:ib + 1],
                                     lhsT=esS[:, ib * 128:(ib + 1) * 128],
                                     rhs=onesc_sb)
                nc.vector.reciprocal(out=r_col, in_=scol)

            def s2():
                r8_ps = ps.tile([8, C], f32, tag="ps", name=f"r8ps{half}")
                nc.tensor.transpose(out=r8_ps, in_=r_col, identity=ident_sb)
                nc.vector.tensor_copy(out=r8_sb, in_=r8_ps)

            def s3():
                rbc = ps.tile([C, HALF], f32, tag="ps", name=f"rbc{half}")
                for k2 in range(8):
                    nc.tensor.matmul(out=rbc[:, k2 * 128:(k2 + 1) * 128],
                                     lhsT=sel8_sb[:, k2 * C:(k2 + 1) * C],
                                     rhs=r8_sb)
                # normalize straight out of PSUM (no ACT copy of rbc)
                nc.vector.tensor_mul(out=onrm, in0=oc16, in1=rbc)

            def s4():
                op_ps = ps.tile([C, HALF], f32, tag="ps", name=f"op{half}")
                for k in range(2):
                    nc.tensor.matmul(out=op_ps[:, k * 512:(k + 1) * 512],
                                     lhsT=wo_sb,
                                     rhs=onrm[:, k * 512:(k + 1) * 512])
                box["op"] = op_ps

            def s5():
                op_ps = box["op"]
                for k in range(2):
                    y_sb = ypool.tile([C, 512], f16, name=f"y{half}_{k}")
                    nc.vector.scalar_tensor_tensor(
                        out=y_sb, in0=op_ps[:, k * 512:(k + 1) * 512],
                        scalar=obias_sb,
                        in1=x16[:, q0 + k * 512:q0 + (k + 1) * 512],
                        op0=OP.add, op1=OP.add)
                    nc.sync.dma_start(
                        out=y_d.ap()[:, q0 + k * 512:q0 + (k + 1) * 512],
                        in_=y_sb)

            return [s0, s1, s2, s3, s4, s5]

        nop = lambda: None
        proj_rest = [lambda: proj_k(1), lambda: proj_v(1),
                     lambda: proj_k(2), lambda: proj_v(2), lambda: proj_q(1),
                     lambda: proj_k(3), lambda: proj_v(3)]
        h0 = emit_half(0, proj_rest + [nop] * 9)
        ep0 = epilogue_steps(0, *h0)
        h1 = emit_half(1, ep0 + [nop] * 10)
        for s in epilogue_steps(1, *h1):
            s()
            # keep the PE HAM window open through the serial tail
            wpt = ps.tile([C, 1024], f32, tag="ps", name="warmt")
            nc.tensor.matmul(out=wpt[:, 0:512], lhsT=wz[:, 0:C], rhs=wz)

        for p in (pot, ps, ypool, accs, spool, epool, work, big, consts):
            p.release()

    nc.compile()
    return nc


def _get_nc():
    global _NC
    if _NC is None:
        _NC = _build_program()
    return _NC


def _make_packs(inputs):
    wq = (np.asarray(inputs["Wq"], dtype=np.float32) * (C ** -0.5)).astype(np.float16)
    wk = np.asarray(inputs["Wk"], dtype=np.float32).astype(np.float16)
    wv = np.asarray(inputs["Wv"], dtype=np.float32).astype(np.float16)
    wo = np.asarray(inputs["Wo"], dtype=np.float32).astype(np.float16)
    gmap = np.zeros((C, 32), np.float16)
    for c in range(C):
        gmap[c, c // 4] = 0.25  # group-average fold
    gmapt = np.zeros((32, C), np.float16)
    for c in range(C):
        gmapt[c // 4, c] = 1.0
    wpack = np.zeros((C, _WPACK_W), np.float16)
    wpack[:, _WQ:_WQ + C] = wq
    wpack[:, _WK:_WK + C] = wk
    wpack[:, _WV:_WV + C] = wv
    wpack[:, _WO:_WO + C] = wo
    wpack[:, _GMAP:_GMAP + 32] = gmap
    wpack[:, _ONESC:_ONESC + 1] = 1.0
    wpack[0:32, _GMAPT:_GMAPT + C] = gmapt
    for k in range(8):
        wpack[k, _SEL8 + k * C:_SEL8 + (k + 1) * C] = 1.0
    fpack = np.zeros((C, _FPACK_W), np.float32)
    fpack[:, _NW] = np.asarray(inputs["norm_w"], dtype=np.float32)
    fpack[:, _NWN] = -np.asarray(inputs["norm_w"], dtype=np.float32)
    fpack[:, _NB] = np.asarray(inputs["norm_b"], dtype=np.float32)
    fpack[:, _IDENT:_IDENT + C] = np.eye(C, dtype=np.float32)
    return wpack, fpack


def _make_in_maps(inputs):
    x = np.asarray(inputs["x"], dtype=np.float32).astype(np.float16)
    B = x.shape[0]
    xf = x.reshape(B, C, HW)
    wpack, fpack = _make_packs(inputs)
    in_maps = []
    for core in range(N_CORES):
        b, s = core // 2, core % 2
        xb = xf[b]
        if s == 1:
            xb = np.concatenate([xb[:, NQ:], xb[:, :NQ]], axis=1)
        in_maps.append({
            "x": np.ascontiguousarray(xb),
            "wpack": wpack, "fpack": fpack,
        })
    return in_maps


def kernel(**inputs):
    from concourse.bass_utils import run_bass_kernel_spmd

    nc = _get_nc()
    in_maps = _make_in_maps(inputs)
    res = run_bass_kernel_spmd(nc, in_maps, list(range(N_CORES)))
    x = np.asarray(inputs["x"], dtype=np.float32)
    B, _, H, W = x.shape
    out = np.empty((B, C, HW), np.float32)
    for core in range(N_CORES):
        b, s = core // 2, core % 2
        out[b, :, s * NQ:(s + 1) * NQ] = res.results[core]["y"].astype(np.float32)
    return out.reshape(B, C, H, W)


# revision 29
# speedup vs baseline: 1.2177x; 1.2177x over previous
"""AttnBlock (GroupNorm + single-head self-attention + residual) on 8 NeuronCores.

Sharding: data-parallel over B (4 batches) x sequence-parallel over query
rows (2 halves of H*W=4096) = 8 shards, one per core.  Each core loads its
batch's full x[b] as [C=128, HW=4096] fp16 (channels on partitions), with
the spatial columns rotated so the core's query half is cols [0:2048)
(attention is permutation-invariant over keys so K/V order is irrelevant).

GroupNorm is folded into the projections: h = A*x + B (per-channel affine
from the group statistics), so only the [128,128] weight scalings depend on
the statistics.  The k-bias is dropped entirely: it shifts every key score
of a given query by the same amount, which softmax cancels.  The V-bias is
folded into an output bias (softmax rows sum to 1).

Main loop: the core's 2048 queries run as two sequential 1024-query passes,
which shrinks the PV accumulator to 2 PSUM banks and leaves three
independent [128,1024] score slots (2 banks each) to triple-buffer the
PE->exp pipeline.  exp work is split three ways: ScalarE exponentiates 3 of
every 4 key blocks ((1024+352)/1.2 ~ 1.1us each); VectorE handles the
fourth with a clamped Schraudolph bit-trick exp (t = A*s + B computed in
fp16, clamped at 0, truncated to int16, bitcast to fp16 ~ exp(s-8) within
+-3.6%); the softmax denominator accumulates in fp16 on two independent
engine-local chains (VectorE for 2 of 3 blocks, GpSimd for the third) so no
cross-engine semaphore hop sits on an accumulation chain.  Each half's
softmax epilogue (column-reduce via PE, reciprocal, selector-matmul
broadcast, normalize straight out of PSUM, Wo projection, residual)
overlaps the other half's loop.
"""

import numpy as np

C = 128
HW = 4096
NQ = 2048  # queries per core
HALF = 1024  # queries per sequential pass
JB = 32  # key blocks of 128
EXP_BIAS = -8.0
EPS = 1e-5
N_CORES = 8
N_WARM = 10  # dummy matmuls to lift the PE HAM clock-gate before real work

# Schraudolph fp16 exp: bitcast(int16(max(SCH_A*s + SCH_B, 0))) ~ exp(s-8)
SCH_A = 1024.0 / float(np.log(2.0))
SCH_B = 15360.0 - 46.0 + SCH_A * EXP_BIAS
SCH_AINV = 1.0 / SCH_A

# wpack (f16) column offsets
_WQ, _WK, _WV, _WO = 0, 128, 256, 384
_GMAP, _ONESC = 512, 544
_GMAPT, _SEL8 = 546, 674
_WPACK_W = 674 + 8 * 128
# fpack (f32) column offsets
_NW, _NWN, _NB, _IDENT = 0, 1, 2, 3
_FPACK_W = 131

_NC = None


def _pin_activation_tables():
    """Restrict the table-load chooser to natural_log_exp_and_others so the
    kernel's ACT stream (copy/identity/ln/exp) needs a single table load."""
    from concourse.hw_specs import get_activation_tables
    tabs = get_activation_tables("gen3")
    for name in list(tabs.keys()):
        if name != "natural_log_exp_and_others":
            tabs[name] = set()


def _build_program():
    import concourse.bacc as bacc
    import concourse.tile as tile
    from concourse import mybir

    f32 = mybir.dt.float32
    f16 = mybir.dt.float16
    i16 = mybir.dt.int16
    AF = mybir.ActivationFunctionType
    OP = mybir.AluOpType

    nc = bacc.Bacc("TRN2", target_bir_lowering=False, debug=False,
                   num_devices=N_CORES)
    try:
        _pin_activation_tables()
    except Exception:
        pass

    x_d = nc.declare_dram_parameter("x", [C, HW], f16, isOutput=False)
    wpack_d = nc.declare_dram_parameter("wpack", [C, _WPACK_W], f16,
                                        isOutput=False)
    fpack_d = nc.declare_dram_parameter("fpack", [C, _FPACK_W], f32,
                                        isOutput=False)
    y_d = nc.declare_dram_parameter("y", [C, NQ], f16, isOutput=True)

    with tile.TileContext(nc) as tc:
        consts = tc.alloc_tile_pool(name="consts", bufs=1)
        big = tc.alloc_tile_pool(name="big", bufs=1)
        work = tc.alloc_tile_pool(name="work", bufs=3)
        epool = tc.alloc_tile_pool(name="epool", bufs=4)   # [C,1024] E tiles
        spool = tc.alloc_tile_pool(name="spool", bufs=2)   # Schraudolph tmp
        accs = tc.alloc_tile_pool(name="accs", bufs=1)
        ypool = tc.alloc_tile_pool(name="ypool", bufs=4)
        # PSUM: 8 banks = 3 score/scratch slots (2 each) + oT (2)
        ps = tc.alloc_tile_pool(name="ps", bufs=3, space="PSUM")
        pot = tc.alloc_tile_pool(name="pot", bufs=1, space="PSUM")

        # ---- input DMAs first: x in 4 chunks of 1024 cols (2KB/partition
        # lines), one per engine ring for queue-level parallelism.
        x16 = big.tile([C, HW], f16)
        for ch, eng in enumerate((nc.sync, nc.scalar, nc.gpsimd, nc.sync)):
            eng.dma_start(out=x16[:, ch * 1024:(ch + 1) * 1024],
                          in_=x_d.ap()[:, ch * 1024:(ch + 1) * 1024])
        wpack_sb = consts.tile([C, _WPACK_W], f16)
        nc.gpsimd.dma_start(out=wpack_sb, in_=wpack_d.ap())
        fpack_sb = consts.tile([C, _FPACK_W], f32)
        nc.gpsimd.dma_start(out=fpack_sb, in_=fpack_d.ap())
        wq_sb = wpack_sb[:, _WQ:_WQ + C]
        wk_sb = wpack_sb[:, _WK:_WK + C]
        wv_sb = wpack_sb[:, _WV:_WV + C]
        wo_sb = wpack_sb[:, _WO:_WO + C]
        gmap_sb = wpack_sb[:, _GMAP:_GMAP + 32]
        onesc_sb = wpack_sb[:, _ONESC:_ONESC + 1]
        gmapt_sb = wpack_sb[0:32, _GMAPT:_GMAPT + C]
        sel8_sb = wpack_sb[0:8, _SEL8:_SEL8 + 8 * C]
        nw_sb = fpack_sb[:, _NW:_NW + 1]
        nwn_sb = fpack_sb[:, _NWN:_NWN + 1]
        nb_sb = fpack_sb[:, _NB:_NB + 1]
        ident_sb = fpack_sb[:, _IDENT:_IDENT + C]
        eps_sb = consts.tile([32, 1], f32)
        nc.vector.memset(eps_sb, EPS)
        ebias_sb = consts.tile([C, 1], f32)
        nc.vector.memset(ebias_sb, EXP_BIAS)

        # ---- PE warmup: back-to-back dummy matmuls so the HAM clock-gate
        # reaches K=8/8 (2.4 GHz) before the real matmul stream starts.
        wz = consts.tile([C, 512], f16)
        nc.vector.memset(wz, 0.0)
        warm_ps = ps.tile([C, 1024], f32, tag="ps", name="warm0")
        for _ in range(N_WARM):
            nc.tensor.matmul(out=warm_ps[:, 0:512], lhsT=wz[:, 0:C],
                             rhs=wz)

        # ---- GroupNorm stats.  Chunk-gated dummy matmul bursts keep the
        # HAM activity window busy through the DMA/stats phase.
        stats = work.tile([C, 8, 6], f32)
        for ch in range(4):
            for h in range(2):
                nc.vector.bn_stats(
                    out=stats[:, 2 * ch + h, :],
                    in_=x16[:, ch * 1024 + h * 512:ch * 1024 + (h + 1) * 512])
            wp = ps.tile([C, 1024], f32, tag="ps", name=f"warmx{ch}")
            for k in range(4):
                nc.tensor.matmul(out=wp[:, 0:512],
                                 lhsT=x16[:, ch * 1024:ch * 1024 + C],
                                 rhs=x16[:, ch * 1024:ch * 1024 + 512])
        mv = work.tile([C, 2], f32)
        nc.vector.bn_aggr(out=mv, in_=stats)
        # spack = (mean, E[x^2]) packed f16; E[x^2] = mean^2 + var in one op
        spack = work.tile([C, 2], f16)
        nc.vector.tensor_scalar(out=spack[:, 1:2], in0=mv[:, 0:1],
                                scalar1=mv[:, 0:1], scalar2=mv[:, 1:2],
                                op0=OP.mult, op1=OP.add)
        nc.vector.tensor_copy(out=spack[:, 0:1], in_=mv[:, 0:1])
        # gmap carries the 0.25 group-average fold
        gsum = ps.tile([32, 2], f32, tag="ps", name="gsum")
        nc.tensor.matmul(out=gsum, lhsT=gmap_sb, rhs=spack)
        # keep HAM busy across the bn_aggr -> gsum hop (behind gsum in the
        # PE FIFO so the filler never delays the chain)
        wpa = ps.tile([C, 1024], f32, tag="ps", name="warma")
        for _ in range(4):
            nc.tensor.matmul(out=wpa[:, 0:512], lhsT=wz[:, 0:C], rhs=wz)
        # -gvar = gm^2 - gex2 in one op; rstd = exp(-0.5*ln(eps - (-gvar)))
        ngvar = work.tile([32, 1], f32)
        nc.vector.tensor_scalar(out=ngvar, in0=gsum[:, 0:1],
                                scalar1=gsum[:, 0:1], scalar2=gsum[:, 1:2],
                                op0=OP.mult, op1=OP.subtract)
        gln = work.tile([32, 1], f32)
        nc.scalar.activation(out=gln, in_=ngvar, func=AF.Ln, bias=eps_sb,
                             scale=-1.0)
        grs = work.tile([32, 1], f32)
        nc.scalar.activation(out=grs, in_=gln, func=AF.Exp, scale=-0.5)
        gpack = work.tile([32, 2], f16)
        nc.vector.tensor_copy(out=gpack[:, 0:1], in_=gsum[:, 0:1])
        nc.vector.tensor_copy(out=gpack[:, 1:2], in_=grs)
        cstat = ps.tile([C, 2], f32, tag="ps", name="cstat")
        nc.tensor.matmul(out=cstat, lhsT=gmapt_sb, rhs=gpack)
        # filler burst behind cstat keeps HAM at 8/8 through the chain
        wpd = ps.tile([C, 1024], f32, tag="ps", name="warmd")
        for _ in range(4):
            nc.tensor.matmul(out=wpd[:, 0:512], lhsT=wz[:, 0:C], rhs=wz)
        affA = work.tile([C, 1], f32)
        nc.vector.tensor_mul(out=affA, in0=cstat[:, 1:2], in1=nw_sb)
        affAn = work.tile([C, 1], f32)
        nc.vector.tensor_mul(out=affAn, in0=cstat[:, 1:2], in1=nwn_sb)
        # affB = nb + cstat0 * (-affA)
        affB = work.tile([C, 1], f32)
        nc.vector.scalar_tensor_tensor(out=affB, in0=cstat[:, 0:1],
                                       scalar=affAn, in1=nb_sb,
                                       op0=OP.mult, op1=OP.add)
        b16 = work.tile([C, 1], f16)
        nc.vector.tensor_copy(out=b16, in_=affB)

        # ---- fold affine scale into projection weights; biases via tiny MMs
        wqa = consts.tile([C, C], f16)
        nc.vector.tensor_scalar_mul(out=wqa, in0=wq_sb, scalar1=affA)
        wka = consts.tile([C, C], f16)
        nc.vector.tensor_scalar_mul(out=wka, in0=wk_sb, scalar1=affA)
        wva = consts.tile([C, C], f16)
        nc.vector.tensor_scalar_mul(out=wva, in0=wv_sb, scalar1=affA)
        pb = ps.tile([C, 2], f32, tag="ps", name="pb")
        nc.tensor.matmul(out=pb[:, 0:1], lhsT=wq_sb, rhs=b16)
        nc.tensor.matmul(out=pb[:, 1:2], lhsT=wv_sb, rhs=b16)
        qb_sb = work.tile([C, 1], f32)
        nc.vector.tensor_copy(out=qb_sb, in_=pb[:, 0:1])
        vb16 = work.tile([C, 1], f16)
        nc.vector.tensor_copy(out=vb16, in_=pb[:, 1:2])
        pob = ps.tile([C, 1], f32, tag="ps", name="pob")
        nc.tensor.matmul(out=pob, lhsT=wo_sb, rhs=vb16)
        obias_sb = work.tile([C, 1], f32)
        nc.vector.tensor_copy(out=obias_sb, in_=pob)

        # ---- projections in [C,1024] slabs.  Loop-start needs qT half0 +
        # kT slab0 (+ V slab0 shortly after); those go first, the remaining
        # slabs are emitted interleaved with the first loop groups.
        # Evictions: Q on ACT (bias), K on DVE, V on GpSimd.
        qT = big.tile([C, NQ], f16)
        kT = big.tile([C, HW], f16)
        v_sb = big.tile([C, HW], f16)  # col block jb holds V0[j, c] rows

        def proj_q(half):
            qps = ps.tile([C, 1024], f32, tag="ps", name=f"qps{half}")
            for k in range(2):
                nc.tensor.matmul(
                    out=qps[:, k * 512:(k + 1) * 512], lhsT=wqa,
                    rhs=x16[:, half * 1024 + k * 512:half * 1024 + (k + 1) * 512])
            nc.scalar.activation(out=qT[:, half * 1024:(half + 1) * 1024],
                                 in_=qps, func=AF.Identity, bias=qb_sb)

        def proj_k(sl):
            kps = ps.tile([C, 1024], f32, tag="ps", name=f"kps{sl}")
            for k in range(2):
                nc.tensor.matmul(
                    out=kps[:, k * 512:(k + 1) * 512], lhsT=wka,
                    rhs=x16[:, sl * 1024 + k * 512:sl * 1024 + (k + 1) * 512])
            nc.vector.tensor_copy(out=kT[:, sl * 1024:(sl + 1) * 1024],
                                  in_=kps)

        def proj_v(sl):
            vps = ps.tile([C, 1024], f32, tag="ps", name=f"vps{sl}")
            for k in range(8):
                jb = sl * 8 + k
                nc.tensor.matmul(out=vps[:, k * 128:(k + 1) * 128],
                                 lhsT=x16[:, jb * 128:(jb + 1) * 128],
                                 rhs=wva)
            if sl % 2 == 0:
                nc.scalar.copy(out=v_sb[:, sl * 1024:(sl + 1) * 1024],
                               in_=vps)
            else:
                nc.vector.tensor_copy(out=v_sb[:, sl * 1024:(sl + 1) * 1024],
                                      in_=vps)

        proj_q(0)
        proj_k(0)
        proj_v(0)

        # ---- main attention loop: two sequential 1024-query passes, 32 key
        # blocks each, software-pipelined by one block (scores jb+1 issue
        # before PV jb).  exp: jb%4==3 -> VectorE Schraudolph, else ScalarE.
        # denominator chains: jb%3==2 -> GpSimd accumulator, else VectorE
        # (two sub-accumulators to break the latency chain).
        def emit_half(half, interleave):
            q0 = half * HALF
            qs = qT[:, q0:q0 + HALF]
            # acc2 halves hold even-jb / odd-jb partial sums
            acc2 = accs.tile([C, 2 * HALF], f16, name=f"acc2_{half}")
            oT = pot.tile([C, HALF], f32, tag="ot", name=f"oT{half}")
            pend = []
            e2 = None
            for jb in range(JB):
                st = ps.tile([C, HALF], f32, tag="ps", name=f"st{half}_{jb}")
                for k in range(2):
                    nc.tensor.matmul(out=st[:, k * 512:(k + 1) * 512],
                                     lhsT=kT[:, jb * 128:(jb + 1) * 128],
                                     rhs=qs[:, k * 512:(k + 1) * 512])
                if jb % 2 == 0:
                    e2 = epool.tile([C, 2 * HALF], f16, tag="e",
                                    name=f"e{half}_{jb}")
                e_t = e2[:, (jb % 2) * HALF:(jb % 2 + 1) * HALF]
                if jb % 6 == 3:
                    tt = spool.tile([C, HALF], f16, tag="s",
                                    name=f"sch{half}_{jb}")
                    nc.vector.tensor_scalar(out=tt, in0=st,
                                            scalar1=SCH_A, scalar2=SCH_B,
                                            op0=OP.mult, op1=OP.add)
                    nc.vector.tensor_scalar_max(out=e_t.bitcast(i16),
                                                in0=tt, scalar1=0.0)
                else:
                    nc.scalar.activation(out=e_t, in_=st, func=AF.Exp,
                                         bias=ebias_sb)
                # one paired denominator accumulation per two blocks
                if jb % 2 == 1:
                    if jb == 1:
                        nc.vector.tensor_copy(out=acc2, in_=e2)
                    else:
                        nc.vector.tensor_add(out=acc2, in0=acc2, in1=e2)
                # PV runs two blocks behind exp so the PE never waits on it
                if len(pend) == 2:
                    pj, pe = pend.pop(0)
                    for k in range(2):
                        nc.tensor.matmul(
                            out=oT[:, k * 512:(k + 1) * 512],
                            lhsT=v_sb[:, pj * 128:(pj + 1) * 128],
                            rhs=pe[:, k * 512:(k + 1) * 512],
                            start=(pj == 0), stop=(pj == JB - 1))
                pend.append((jb, e_t))
                # previous half's epilogue / projection slabs trickle in
                if interleave and jb % 2 == 1:
                    interleave.pop(0)()
            for pj, pe in pend:
                for k in range(2):
                    nc.tensor.matmul(out=oT[:, k * 512:(k + 1) * 512],
                                     lhsT=v_sb[:, pj * 128:(pj + 1) * 128],
                                     rhs=pe[:, k * 512:(k + 1) * 512],
                                     start=(pj == 0), stop=(pj == JB - 1))
            return acc2, oT

        def epilogue_steps(half, acc2, oT):
            """Return the softmax epilogue as closures; step 0 frees oT."""
            q0 = half * HALF
            esS = work.tile([C, HALF], f16, name=f"esS{half}")
            r_col = work.tile([C, 8], f32, name=f"rcol{half}")
            r8_sb = work.tile([8, C], f16, name=f"r8sb{half}")
            oc16 = work.tile([C, HALF], f16, name=f"oc16_{half}")
            onrm = work.tile([C, HALF], f16, name=f"onrm{half}")
            box = {}

            def s0():
                nc.scalar.copy(out=oc16, in_=oT)  # unnormalized; frees oT
                nc.vector.tensor_add(out=esS, in0=acc2[:, 0:HALF],
                                     in1=acc2[:, HALF:2 * HALF])

            def s1():
                scol = ps.tile([C, 8], f32, tag="ps", name=f"scol{half}")
                for ib in range(8):
                    nc.tensor.matmul(out=scol[:, ib:ib + 1],
                                     lhsT=esS[:, ib * 128:(ib + 1) * 128],
                                     rhs=onesc_sb)
                nc.vector.reciprocal(out=r_col, in_=scol)
                if half == 1:
                    # tail: project the unnormalized output through Wo in
                    # parallel with the denominator chain (column scaling
                    # commutes through the matmul); ACT is idle here
                    opu_ps = ps.tile([C, HALF], f32, tag="ps",
                                     name=f"opu{half}")
                    for k in range(2):
                        nc.tensor.matmul(out=opu_ps[:, k * 512:(k + 1) * 512],
                                         lhsT=wo_sb,
                                         rhs=oc16[:, k * 512:(k + 1) * 512])
                    nc.scalar.copy(out=onrm, in_=opu_ps)  # onrm = op16 here

            def s2():
                r8_ps = ps.tile([8, C], f32, tag="ps", name=f"r8ps{half}")
                nc.tensor.transpose(out=r8_ps, in_=r_col, identity=ident_sb)
                nc.vector.tensor_copy(out=r8_sb, in_=r8_ps)

            def s3():
                rbc = ps.tile([C, HALF], f32, tag="ps", name=f"rbc{half}")
                for k2 in range(8):
                    nc.tensor.matmul(out=rbc[:, k2 * 128:(k2 + 1) * 128],
                                     lhsT=sel8_sb[:, k2 * C:(k2 + 1) * C],
                                     rhs=r8_sb)
                if half == 1:
                    yall = ypool.tile([C, HALF], f16, name=f"yall{half}")
                    nc.vector.tensor_mul(out=yall, in0=onrm, in1=rbc)
                    box["yall"] = yall
                else:
                    # normalize straight out of PSUM (no ACT copy of rbc)
                    nc.vector.tensor_mul(out=onrm, in0=oc16, in1=rbc)

            def s4():
                if half == 1:
                    return
                op_ps = ps.tile([C, HALF], f32, tag="ps", name=f"op{half}")
                for k in range(2):
                    nc.tensor.matmul(out=op_ps[:, k * 512:(k + 1) * 512],
                                     lhsT=wo_sb,
                                     rhs=onrm[:, k * 512:(k + 1) * 512])
                box["op"] = op_ps

            def s5():
                for k in range(2):
                    y_sb = ypool.tile([C, 512], f16, name=f"y{half}_{k}")
                    if half == 1:
                        nc.vector.scalar_tensor_tensor(
                            out=y_sb,
                            in0=box["yall"][:, k * 512:(k + 1) * 512],
                            scalar=obias_sb,
                            in1=x16[:, q0 + k * 512:q0 + (k + 1) * 512],
                            op0=OP.add, op1=OP.add)
                    else:
                        nc.vector.scalar_tensor_tensor(
                            out=y_sb, in0=box["op"][:, k * 512:(k + 1) * 512],
                            scalar=obias_sb,
                            in1=x16[:, q0 + k * 512:q0 + (k + 1) * 512],
                            op0=OP.add, op1=OP.add)
                    nc.sync.dma_start(
                        out=y_d.ap()[:, q0 + k * 512:q0 + (k + 1) * 512],
                        in_=y_sb)

            return [s0, s1, s2, s3, s4, s5]

        nop = lambda: None
        proj_rest = [lambda: proj_k(1), lambda: proj_v(1),
                     lambda: proj_k(2), lambda: proj_v(2), lambda: proj_q(1),
                     lambda: proj_k(3), lambda: proj_v(3)]
        h0 = emit_half(0, proj_rest + [nop] * 9)
        ep0 = epilogue_steps(0, *h0)
        h1 = emit_half(1, ep0 + [nop] * 10)
        for s in epilogue_steps(1, *h1):
            s()
            # keep the PE HAM window open through the serial tail
            wpt = ps.tile([C, 1024], f32, tag="ps", name="warmt")
            nc.tensor.matmul(out=wpt[:, 0:512], lhsT=wz[:, 0:C], rhs=wz)

        for p in (pot, ps, ypool, accs, spool, epool, work, big, consts):
            p.release()

    nc.compile()
    return nc


def _get_nc():
    global _NC
    if _NC is None:
        _NC = _build_program()
    return _NC


def _make_packs(inputs):
    wq = (np.asarray(inputs["Wq"], dtype=np.float32) * (C ** -0.5)).astype(np.float16)
    wk = np.asarray(inputs["Wk"], dtype=np.float32).astype(np.float16)
    wv = np.asarray(inputs["Wv"], dtype=np.float32).astype(np.float16)
    wo = np.asarray(inputs["Wo"], dtype=np.float32).astype(np.float16)
    gmap = np.zeros((C, 32), np.float16)
    for c in range(C):
        gmap[c, c // 4] = 0.25  # group-average fold
    gmapt = np.zeros((32, C), np.float16)
    for c in range(C):
        gmapt[c // 4, c] = 1.0
    wpack = np.zeros((C, _WPACK_W), np.float16)
    wpack[:, _WQ:_WQ + C] = wq
    wpack[:, _WK:_WK + C] = wk
    wpack[:, _WV:_WV + C] = wv
    wpack[:, _WO:_WO + C] = wo
    wpack[:, _GMAP:_GMAP + 32] = gmap
    wpack[:, _ONESC:_ONESC + 1] = 1.0
    wpack[0:32, _GMAPT:_GMAPT + C] = gmapt
    for k in range(8):
        wpack[k, _SEL8 + k * C:_SEL8 + (k + 1) * C] = 1.0
    fpack = np.zeros((C, _FPACK_W), np.float32)
    fpack[:, _NW] = np.asarray(inputs["norm_w"], dtype=np.float32)
    fpack[:, _NWN] = -np.asarray(inputs["norm_w"], dtype=np.float32)
    fpack[:, _NB] = np.asarray(inputs["norm_b"], dtype=np.float32)
    fpack[:, _IDENT:_IDENT + C] = np.eye(C, dtype=np.float32)
    return wpack, fpack


def _make_in_maps(inputs):
    x = np.asarray(inputs["x"], dtype=np.float32).astype(np.float16)
    B = x.shape[0]
    xf = x.reshape(B, C, HW)
    wpack, fpack = _make_packs(inputs)
    in_maps = []
    for core in range(N_CORES):
        b, s = core // 2, core % 2
        xb = xf[b]
        if s == 1:
            xb = np.concatenate([xb[:, NQ:], xb[:, :NQ]], axis=1)
        in_maps.append({
            "x": np.ascontiguousarray(xb),
            "wpack": wpack, "fpack": fpack,
        })
    return in_maps


def kernel(**inputs):
    from concourse.bass_utils import run_bass_kernel_spmd

    nc = _get_nc()
    in_maps = _make_in_maps(inputs)
    res = run_bass_kernel_spmd(nc, in_maps, list(range(N_CORES)))
    x = np.asarray(inputs["x"], dtype=np.float32)
    B, _, H, W = x.shape
    out = np.empty((B, C, HW), np.float32)
    for core in range(N_CORES):
        b, s = core // 2, core % 2
        out[b, :, s * NQ:(s + 1) * NQ] = res.results[core]["y"].astype(np.float32)
    return out.reshape(B, C, H, W)
